# revision 1
# baseline (speedup 1.0000x reference)
"""Trainium2 Bass kernel for nn_CombinedMLPMoEModel (moe_routing).

Strategy (8 NeuronCores, pure data parallel over batch):
- Activations kept transposed on device: [features(partitions), batch(free)].
- Pre-gate MLP chain (p1/p2, m1, m2, m3) runs as 3-pass float32r "split"
  matmuls (hi/lo mantissa splits of both operands) giving fp32-level accuracy
  at 3 cycles/row, so the MoE top-2 routing matches the fp32 reference
  exactly (top-k selection is the only discontinuous op in the model).
- LayerNorm statistics + gate logits + final regression use native fp32
  matmuls (exact); BN layers are folded into the adjacent weights on host.
- Dense MoE: all 8 experts evaluated in bf16 with per-(token,expert) softmax
  weights applied to the expert INPUT (xw = x * w), so expert accumulation
  happens in PSUM across experts; expert bias enters as one extra K=8 matmul
  (lhsT=ex_b, rhs=w^T).
- Fusion head in single-pass float32r.
"""

import sys
import types

sys.path.insert(0, "/opt/trn_rl_repo")

import numpy as np
import ml_dtypes

import concourse.bacc as bacc
import concourse.mybir as mybir
import concourse.tile as tile
from concourse.bass_utils import run_bass_kernel_spmd
from concourse.masks import make_identity

F32 = mybir.dt.float32
F32R = mybir.dt.float32r
BF16 = mybir.dt.bfloat16
AF = mybir.ActivationFunctionType
ALU = mybir.AluOpType
AX = mybir.AxisListType

NCORES = 8
B = 16384
BC = B // NCORES  # 2048 per core
D = 1024
H = 2048
E = 8
EPS = 1e-5
P = 128
CHUNK = 1024  # batch columns processed per device-side chunk
NCHUNK = BC // CHUNK
BT = 512  # matmul moving-dim tile


def _q_f32r(x):
    """Round-half-up to float32r (12 low mantissa bits dropped) - matches HW."""
    x = np.ascontiguousarray(x, dtype=np.float32)
    xi = x.view(np.uint32)
    out = ((xi + np.uint32(1 << 11)) & (np.uint32(0xFFFFFFFF) << np.uint32(12))).view(
        np.float32
    )
    return out.reshape(x.shape).copy()


def _split_f32r(x):
    xh = _q_f32r(x)
    xl = _q_f32r(np.asarray(x, np.float32) - xh)
    return xh, xl


def _bias_tiles(b):
    """[n] -> [128, n//128]: column t = b[t*128:(t+1)*128] (per-partition bias)."""
    n = b.shape[0]
    return np.ascontiguousarray(b.reshape(n // P, P).T, dtype=np.float32)


def _build_program():
    nc = bacc.Bacc(None, target_bir_lowering=False)

    def din(name, shape, dt):
        return nc.dram_tensor(name, list(shape), dt, kind="ExternalInput")

    x1h = din("x1h", [D, BC], F32R)
    x1l = din("x1l", [D, BC], F32R)
    x2h = din("x2h", [D, BC], F32R)
    x2l = din("x2l", [D, BC], F32R)
    p1h = din("p1h", [D, D], F32R)
    p1l = din("p1l", [D, D], F32R)
    p2h = din("p2h", [D, D], F32R)
    p2l = din("p2l", [D, D], F32R)
    m1h = din("m1h", [2 * D, H], F32R)
    m1l = din("m1l", [2 * D, H], F32R)
    m2h = din("m2h", [H, H], F32R)
    m2l = din("m2l", [H, H], F32R)
    m3h = din("m3h", [H, 2 * D], F32R)
    m3l = din("m3l", [H, 2 * D], F32R)
    pb1 = din("pb1", [P, D // P], F32)
    pb2 = din("pb2", [P, D // P], F32)
    mb1 = din("mb1", [P, H // P], F32)
    mb2 = din("mb2", [P, H // P], F32)
    mb3 = din("mb3", [P, 2 * D // P], F32)
    lng = din("lng", [P, 2 * D // P], F32)
    lnb = din("lnb", [P, 2 * D // P], F32)
    gwt = din("gwt", [D, E], F32)
    gbb = din("gbb", [P, E], F32)
    exw = din("exw", [E, D, D], BF16)
    exb = din("exb", [E, D], BF16)
    fwt = din("fwt", [2 * D, D], BF16)
    fb = din("fb", [P, D // P], F32)
    rwt = din("rwt", [D, 1], F32)
    rb = din("rb", [1, 1], F32)

    out = nc.dram_tensor("out", [1, BC], F32, kind="ExternalOutput")

    from contextlib import ExitStack

    with tile.TileContext(nc) as tc, ExitStack() as ctx:
        const = ctx.enter_context(tc.tile_pool(name="const", bufs=1))
        act = ctx.enter_context(tc.tile_pool(name="act", bufs=1))
        tp = ctx.enter_context(tc.tile_pool(name="tp", bufs=2))
        tpk = ctx.enter_context(tc.tile_pool(name="tpk", bufs=2))
        wp = ctx.enter_context(tc.tile_pool(name="wp", bufs=2))
        sp = ctx.enter_context(tc.tile_pool(name="sp", bufs=4))
        rows = ctx.enter_context(tc.tile_pool(name="rows", bufs=4))
        ps = ctx.enter_context(tc.tile_pool(name="ps", bufs=4, space="PSUM"))
        dram = ctx.enter_context(tc.tile_pool(name="dram", bufs=4, space="DRAM"))

        # constants
        ones_col = const.tile([P, 1], F32)
        nc.vector.memset(ones_col, 1.0)
        eps_row = const.tile([1, 1], F32)
        nc.vector.memset(eps_row, EPS)
        ident_bf = const.tile([P, P], BF16)
        make_identity(nc, ident_bf)
        gwt_sb = const.tile([P, D // P, E], F32)
        nc.sync.dma_start(gwt_sb, gwt.rearrange("(kt p) e -> p kt e", p=P))
        gbb_sb = const.tile([P, E], F32)
        nc.sync.dma_start(gbb_sb, gbb[:, :])
        lng_sb = const.tile([P, 2 * D // P], F32)
        nc.sync.dma_start(lng_sb, lng[:, :])
        lnb_sb = const.tile([P, 2 * D // P], F32)
        nc.sync.dma_start(lnb_sb, lnb[:, :])
        exb_sb = const.tile([E, D], BF16)
        nc.sync.dma_start(exb_sb, exb[:, :])
        rwt_sb = const.tile([P, D // P, 1], F32)
        nc.sync.dma_start(rwt_sb, rwt.rearrange("(kt p) o -> p kt o", p=P))
        rb_sb = const.tile([1, 1], F32)
        nc.sync.dma_start(rb_sb, rb[:, :])
        bias_sbs = {}
        for name, t, nt in [
            ("pb1", pb1, D // P),
            ("pb2", pb2, D // P),
            ("mb1", mb1, H // P),
            ("mb2", mb2, H // P),
            ("mb3", mb3, 2 * D // P),
            ("fb", fb, D // P),
        ]:
            s = const.tile([P, nt], F32, name=f"bias_{name}", tag=f"bias_{name}")
            nc.sync.dma_start(s, t[:, :])
            bias_sbs[name] = s

        wt_dram = [
            dram.tile([E, BC], BF16, name=f"wt_dram{i}", tag=f"wt_dram{i}")
            for i in range(2)
        ]

        def split_layer(
            in_provider,
            w_hi,
            w_lo,
            kt,
            ot,
            bias_sb,
            relu,
            out_tile,
            out_ft_off,
            ccols,
        ):
            """One split-3 f32r layer: out[:, ot tiles, ccols] over CHUNK cols.

            in_provider(k, g) -> (kh, kl) SBUF tiles [P, CHUNK] f32r.
            w_hi/w_lo: DRAM [kt*P, ot*P]. psum: groups of 4 dout tiles,
            each [P, CHUNK] (2 banks).
            """
            ngroups = ot // 4
            for g in range(ngroups):
                psums = [
                    ps.tile([P, CHUNK], F32, name=f"mmps{i}", tag="ps")
                    for i in range(4)
                ]
                for k in range(kt):
                    kh, kl = in_provider(k, g)
                    wg_h = wp.tile([P, 4 * P], F32R, tag="wg_h")
                    nc.scalar.dma_start(
                        wg_h, w_hi[k * P : (k + 1) * P, g * 4 * P : (g + 1) * 4 * P]
                    )
                    wg_l = wp.tile([P, 4 * P], F32R, tag="wg_l")
                    nc.scalar.dma_start(
                        wg_l, w_lo[k * P : (k + 1) * P, g * 4 * P : (g + 1) * 4 * P]
                    )
                    for dg in range(4):
                        wh_t = wg_h[:, dg * P : (dg + 1) * P]
                        wl_t = wg_l[:, dg * P : (dg + 1) * P]
                        for bt in range(CHUNK // BT):
                            sl = slice(bt * BT, (bt + 1) * BT)
                            first = k == 0
                            last = k == kt - 1
                            nc.tensor.matmul(
                                psums[dg][:, sl], wh_t, kh[:, sl],
                                start=first, stop=False,
                            )
                            nc.tensor.matmul(
                                psums[dg][:, sl], wh_t, kl[:, sl],
                                start=False, stop=False,
                            )
                            nc.tensor.matmul(
                                psums[dg][:, sl], wl_t, kh[:, sl],
                                start=False, stop=last,
                            )
                for dg in range(4):
                    dout = g * 4 + dg
                    for bt in range(CHUNK // BT):
                        sl = slice(bt * BT, (bt + 1) * BT)
                        nc.scalar.activation(
                            out_tile[:, out_ft_off + dout, sl],
                            psums[dg][:, sl],
                            AF.Relu if relu else AF.Identity,
                            bias=bias_sb[:, dout : dout + 1],
                            scale=1.0,
                        )

        def sbuf_split_provider(src_tile):
            """Split f32 activation tiles [P, ft, CHUNK] on the fly."""

            def provider(k, g):
                kh = tpk.tile([P, CHUNK], F32R, tag="kh")
                nc.gpsimd.dma_start(kh, src_tile[:, k, :])
                kl = tpk.tile([P, CHUNK], F32R, tag="kl")
                nc.vector.tensor_tensor(
                    kl, src_tile[:, k, :], kh.bitcast(F32), ALU.subtract
                )
                return kh, kl

            return provider

        def dram_pair_provider(xh_d, xl_d, ccols):
            def provider(k, g):
                kh = tpk.tile([P, CHUNK], F32R, tag="kh")
                nc.sync.dma_start(kh, xh_d[k * P : (k + 1) * P, ccols])
                kl = tpk.tile([P, CHUNK], F32R, tag="kl")
                nc.sync.dma_start(kl, xl_d[k * P : (k + 1) * P, ccols])
                return kh, kl

            return provider

        for c in range(NCHUNK):
            ccols = slice(c * CHUNK, (c + 1) * CHUNK)

            # ---- projections -> h [2048 feats, CHUNK] f32
            h = act.tile([P, 16, CHUNK], F32, tag="h")
            split_layer(
                dram_pair_provider(x1h, x1l, ccols), p1h, p1l, D // P, D // P,
                bias_sbs["pb1"], False, h, 0, ccols,
            )
            split_layer(
                dram_pair_provider(x2h, x2l, ccols), p2h, p2l, D // P, D // P,
                bias_sbs["pb2"], False, h, 8, ccols,
            )

            # ---- m1, m2 (relu), m3 (no relu)
            h1 = act.tile([P, 16, CHUNK], F32, tag="h1")
            split_layer(
                sbuf_split_provider(h), m1h, m1l, 16, 16,
                bias_sbs["mb1"], True, h1, 0, ccols,
            )
            h2 = act.tile([P, 16, CHUNK], F32, tag="h")
            split_layer(
                sbuf_split_provider(h1), m2h, m2l, 16, 16,
                bias_sbs["mb2"], True, h2, 0, ccols,
            )
            a = act.tile([P, 16, CHUNK], F32, tag="h1")
            split_layer(
                sbuf_split_provider(h2), m3h, m3l, 16, 16,
                bias_sbs["mb3"], False, a, 0, ccols,
            )

            # ---- LayerNorm over the 2048 features (partition axis, via matmul)
            xm = act.tile([P, 16, CHUNK], F32, tag="h")
            for bt in range(CHUNK // BT):
                sl = slice(bt * BT, (bt + 1) * BT)
                ps_ln = ps.tile([P, CHUNK], F32, name="ps_ln", tag="ps")
                ps_sum = ps_ln[0:1, 0:BT]
                ps_sq = ps_ln[0:1, BT : 2 * BT]
                for ft in range(16):
                    at = a[:, ft, sl]
                    sq = tp.tile([P, BT], F32, tag="lnt")
                    nc.vector.tensor_tensor(sq, at, at, ALU.mult)
                    nc.tensor.matmul(
                        ps_sum, ones_col, at, start=(ft == 0), stop=(ft == 15)
                    )
                    nc.tensor.matmul(
                        ps_sq, ones_col, sq, start=(ft == 0), stop=(ft == 15)
                    )
                mean = rows.tile([1, BT], F32, tag="lnrow")
                nc.vector.tensor_scalar(mean, ps_sum, 1.0 / (2 * D), None, ALU.mult)
                e2 = rows.tile([1, BT], F32, tag="lnrow")
                nc.vector.tensor_scalar(e2, ps_sq, 1.0 / (2 * D), None, ALU.mult)
                nm2 = rows.tile([1, BT], F32, tag="lnrow")
                nc.vector.scalar_tensor_tensor(
                    nm2, mean, -1.0, mean, ALU.mult, ALU.mult
                )  # -mean^2
                var = rows.tile([1, BT], F32, tag="lnrow")
                nc.vector.tensor_tensor(var, e2, nm2, ALU.add)
                sd = rows.tile([1, BT], F32, tag="lnrow")
                nc.scalar.activation(sd, var, AF.Sqrt, bias=eps_row, scale=1.0)
                rstd = rows.tile([1, BT], F32, tag="lnrow")
                nc.vector.reciprocal(rstd, sd)
                trow = rows.tile([1, BT], F32, tag="lnrow")
                nc.vector.scalar_tensor_tensor(
                    trow, mean, -1.0, rstd, ALU.mult, ALU.mult
                )
                # broadcast via DRAM round-trip
                s_d = dram.tile([1, BT], F32, name="s_d", tag="s_d")
                t_d = dram.tile([1, BT], F32, name="t_d", tag="t_d")
                nc.sync.dma_start(s_d[:], rstd)
                nc.sync.dma_start(t_d[:], trow)
                sbc = tp.tile([P, BT], F32, tag="sbc")
                nc.gpsimd.dma_start(sbc, s_d[0:1, :].to_broadcast([P, BT]))
                tbc = tp.tile([P, BT], F32, tag="tbc")
                nc.gpsimd.dma_start(tbc, t_d[0:1, :].to_broadcast([P, BT]))
                for ft in range(16):
                    t1 = tp.tile([P, BT], F32, tag="lnt")
                    nc.vector.tensor_tensor(t1, a[:, ft, sl], sbc, ALU.mult)
                    nc.vector.tensor_tensor(t1, t1, tbc, ALU.add)
                    nc.vector.tensor_scalar(
                        xm[:, ft, sl], t1,
                        lng_sb[:, ft : ft + 1], lnb_sb[:, ft : ft + 1],
                        ALU.mult, ALU.add,
                    )

            # ---- gate + top2 softmax weights (fp32, exact routing)
            wts_sb = [None, None]
            for br in range(2):
                wts = act.tile([E, CHUNK], BF16, tag=f"wts{br}")
                wts_sb[br] = wts
                for bs in range(CHUNK // P):
                    bsl = slice(bs * P, (bs + 1) * P)
                    ps_g = ps.tile([P, CHUNK], F32, name="ps_g", tag="ps")
                    psg = ps_g[:, 0:E]
                    for k in range(8):
                        nc.tensor.matmul(
                            psg, xm[:, br * 8 + k, bsl], gwt_sb[:, k, :],
                            start=(k == 0), stop=(k == 7),
                        )
                    lg = sp.tile([P, E], F32, tag="lg")
                    nc.vector.scalar_tensor_tensor(
                        lg, psg, 1.0, gbb_sb, ALU.mult, ALU.add
                    )
                    mx1 = sp.tile([P, 1], F32, tag="mx1")
                    nc.vector.reduce_max(mx1, lg, axis=AX.X)
                    cmp = sp.tile([P, E], F32, tag="cmp")
                    nc.vector.tensor_scalar(cmp, lg, mx1, -1e30, ALU.is_ge, ALU.mult)
                    masked = sp.tile([P, E], F32, tag="masked")
                    nc.vector.tensor_tensor(masked, lg, cmp, ALU.add)
                    mx2 = sp.tile([P, 1], F32, tag="mx2")
                    nc.vector.reduce_max(mx2, masked, axis=AX.X)
                    negm1 = sp.tile([P, 1], F32, tag="negm1")
                    nc.vector.tensor_scalar(negm1, mx1, -1.0, None, ALU.mult)
                    ex = sp.tile([P, E], F32, tag="ex")
                    nc.scalar.activation(ex, lg, AF.Exp, bias=negm1, scale=1.0)
                    sel = sp.tile([P, E], F32, tag="sel")
                    nc.vector.tensor_scalar(sel, lg, mx2, None, ALU.is_ge)
                    wu = sp.tile([P, E], F32, tag="wu")
                    nc.vector.tensor_tensor(wu, ex, sel, ALU.mult)
                    den = sp.tile([P, 1], F32, tag="den")
                    nc.vector.reduce_sum(den, wu, axis=AX.X)
                    rec = sp.tile([P, 1], F32, tag="rec")
                    nc.vector.reciprocal(rec, den)
                    wbf = sp.tile([P, E], BF16, tag="wbf")
                    nc.vector.tensor_scalar(wbf, wu, rec, None, ALU.mult)
                    ps_t = ps.tile([P, 2 * CHUNK], BF16, name="ps_t", tag="ps")
                    pst = ps_t[0:E, 0:P]
                    nc.tensor.transpose(pst, wbf, ident_bf)
                    nc.vector.tensor_copy(wts[:, bsl], pst)
                nc.sync.dma_start(wt_dram[br][:, ccols], wts)

            # ---- dense MoE (bf16) fused with the fusion head, per bt stripe
            fust = act.tile([P, 8, CHUNK], F32, tag="h1")
            for bt in range(CHUNK // BT):
                sl = slice(bt * BT, (bt + 1) * BT)
                moebt = act.tile([P, 16, BT], BF16, tag="moebt")
                for br in range(2):
                    mt = [
                        ps.tile([P, CHUNK], F32, name=f"mops{i}", tag="ps")
                        for i in range(4)
                    ]
                    psums = [
                        mt[i // 2][:, (i % 2) * BT : (i % 2 + 1) * BT]
                        for i in range(8)
                    ]
                    for e in range(E):
                        wbc = tp.tile([P, BT], BF16, tag="wbc")
                        nc.gpsimd.dma_start(
                            wbc,
                            wt_dram[br][e : e + 1, c * CHUNK + bt * BT :
                                        c * CHUNK + (bt + 1) * BT].to_broadcast(
                                [P, BT]
                            ),
                        )
                        for k in range(8):
                            xw = tp.tile([P, BT], BF16, tag="xw")
                            nc.vector.tensor_tensor(
                                xw, xm[:, br * 8 + k, sl], wbc, ALU.mult
                            )
                            wexp = wp.tile([P, D], BF16, tag="wexp")
                            nc.scalar.dma_start(
                                wexp, exw[e, k * P : (k + 1) * P, :]
                            )
                            for dout in range(8):
                                nc.tensor.matmul(
                                    psums[dout],
                                    wexp[:, dout * P : (dout + 1) * P],
                                    xw,
                                    start=(e == 0 and k == 0),
                                    stop=False,
                                )
                    for dout in range(8):
                        nc.tensor.matmul(
                            psums[dout],
                            exb_sb[:, dout * P : (dout + 1) * P],
                            wts_sb[br][:, sl],
                            start=False,
                            stop=True,
                        )
                        nc.vector.tensor_copy(
                            moebt[:, br * 8 + dout, :], psums[dout]
                        )
                # fusion head for this bt stripe (bf16)
                ft_t = [
                    ps.tile([P, CHUNK], F32, name=f"fps{i}", tag="ps")
                    for i in range(4)
                ]
                psf = [
                    ft_t[i // 2][:, (i % 2) * BT : (i % 2 + 1) * BT]
                    for i in range(8)
                ]
                for k in range(16):
                    wgf = wp.tile([P, D], BF16, tag="wexp")
                    nc.scalar.dma_start(wgf, fwt[k * P : (k + 1) * P, :])
                    for dout in range(8):
                        nc.tensor.matmul(
                            psf[dout],
                            wgf[:, dout * P : (dout + 1) * P],
                            moebt[:, k, :],
                            start=(k == 0),
                            stop=(k == 15),
                        )
                for dout in range(8):
                    nc.scalar.activation(
                        fust[:, dout, sl], psf[dout], AF.Identity,
                        bias=bias_sbs["fb"][:, dout : dout + 1], scale=1.0,
                    )

            for bt in range(CHUNK // BT):
                sl = slice(bt * BT, (bt + 1) * BT)
                ps_r = ps.tile([P, CHUNK], F32, name="ps_r", tag="ps")
                psr = ps_r[0:1, 0:BT]
                for k in range(8):
                    nc.tensor.matmul(
                        psr, rwt_sb[:, k, :], fust[:, k, sl],
                        start=(k == 0), stop=(k == 7),
                    )
                orow = rows.tile([1, BT], F32, tag="lnrow")
                nc.vector.tensor_scalar(orow, psr, rb_sb[0:1, 0:1], None, ALU.add)
                nc.sync.dma_start(out[0:1, c * CHUNK + bt * BT :
                                      c * CHUNK + (bt + 1) * BT], orow)

    nc.compile()
    return nc


_NC_CACHE = None


def _get_program():
    global _NC_CACHE
    if _NC_CACHE is None:
        _NC_CACHE = _build_program()
    return _NC_CACHE


def _host_prep_shared(inp):
    """Weight folding/transposition/splitting shared across all cores."""
    f = np.float32
    g1 = np.asarray(inp["bn1_g"], f) / np.sqrt(f(1.0) + f(EPS))
    g2 = np.asarray(inp["bn2_g"], f) / np.sqrt(f(1.0) + f(EPS))
    gf = np.asarray(inp["bnf_g"], f) / np.sqrt(f(1.0) + f(EPS))

    m1w = (np.asarray(inp["m1_w"], f) * g1[:, None]).T  # [2D, H]
    m1b = np.asarray(inp["m1_b"], f) * g1 + np.asarray(inp["bn1_b"], f)
    m2w = (np.asarray(inp["m2_w"], f) * g2[:, None]).T  # [H, H]
    m2b = np.asarray(inp["m2_b"], f) * g2 + np.asarray(inp["bn2_b"], f)
    fww = (np.asarray(inp["fus_w"], f) * gf[:, None]).T  # [2D, D]
    fbb = np.asarray(inp["fus_b"], f) * gf + np.asarray(inp["bnf_b"], f)

    shared = {}
    for nm, w in [
        ("p1", np.asarray(inp["p1_w"], f).T),
        ("p2", np.asarray(inp["p2_w"], f).T),
        ("m1", m1w),
        ("m2", m2w),
        ("m3", np.asarray(inp["m3_w"], f).T),
    ]:
        hi, lo = _split_f32r(np.ascontiguousarray(w))
        shared[nm + "h"] = hi
        shared[nm + "l"] = lo
    shared["pb1"] = _bias_tiles(np.asarray(inp["p1_b"], f))
    shared["pb2"] = _bias_tiles(np.asarray(inp["p2_b"], f))
    shared["mb1"] = _bias_tiles(m1b)
    shared["mb2"] = _bias_tiles(m2b)
    shared["mb3"] = _bias_tiles(np.asarray(inp["m3_b"], f))
    shared["lng"] = _bias_tiles(np.asarray(inp["ln_g"], f))
    shared["lnb"] = _bias_tiles(np.asarray(inp["ln_b"], f))
    shared["gwt"] = np.ascontiguousarray(np.asarray(inp["gate_w"], f).T)
    shared["gbb"] = np.ascontiguousarray(
        np.broadcast_to(np.asarray(inp["gate_b"], f)[None, :], (P, E))
    )
    exw = np.asarray(inp["ex_w"], f)  # [E, out, in]
    shared["exw"] = np.ascontiguousarray(
        np.transpose(exw, (0, 2, 1))
    ).astype(ml_dtypes.bfloat16)
    shared["exb"] = np.asarray(inp["ex_b"], f).astype(ml_dtypes.bfloat16)
    shared["fwt"] = np.ascontiguousarray(fww).astype(ml_dtypes.bfloat16)
    shared["fb"] = _bias_tiles(fbb)
    shared["rwt"] = np.ascontiguousarray(np.asarray(inp["reg_w"], f).T)
    shared["rb"] = np.asarray(inp["reg_b"], f).reshape(1, 1)
    return shared


def kernel(**inputs):
    nc = _get_program()
    shared = _host_prep_shared(inputs)

    x1 = np.asarray(inputs["x1"], np.float32)
    x2 = np.asarray(inputs["x2"], np.float32)

    in_maps = []
    for core in range(NCORES):
        rows_sl = slice(core * BC, (core + 1) * BC)
        x1t = np.ascontiguousarray(x1[rows_sl].T)  # [D, BC]
        x2t = np.ascontiguousarray(x2[rows_sl].T)
        x1th, x1tl = _split_f32r(x1t)
        x2th, x2tl = _split_f32r(x2t)
        m = dict(shared)
        m["x1h"], m["x1l"] = x1th, x1tl
        m["x2h"], m["x2l"] = x2th, x2tl
        in_maps.append(m)

    kwargs = {}
    if globals().get("TRACE"):
        kwargs["trace"] = True
    res = run_bass_kernel_spmd(nc, in_maps, core_ids=list(range(NCORES)), **kwargs)
    globals()["_LAST_RESULT"] = res
    out = np.concatenate([r["out"][0] for r in res.results])
    return out.reshape(B, 1).astype(np.float32)


if __name__ == "__main__":
    rng = np.random.default_rng(0)
    fake = {"x1": rng.standard_normal((B, D), dtype=np.float32)}
    print("kernel module loaded; use test.py to validate")



# revision 14
# speedup vs baseline: 1.0084x; 1.0084x over previous
"""Trainium2 Bass kernel for nn_CombinedMLPMoEModel (moe_routing).

Strategy (8 NeuronCores, pure data parallel over batch):
- Activations kept transposed on device: [features(partitions), batch(free)].
- Pre-gate MLP chain (p1/p2, m1, m2, m3) runs as 3-pass float32r "split"
  matmuls (hi/lo mantissa splits of both operands) giving fp32-level accuracy
  at 3 cycles/row, so the MoE top-2 routing matches the fp32 reference
  exactly (top-k selection is the only discontinuous op in the model).
- LayerNorm statistics + gate logits + final regression use native fp32
  matmuls (exact); BN layers are folded into the adjacent weights on host.
- Dense MoE: all 8 experts evaluated in bf16 with per-(token,expert) softmax
  weights applied to the expert INPUT (xw = x * w), so expert accumulation
  happens in PSUM across experts; expert bias enters as one extra K=8 matmul
  (lhsT=ex_b, rhs=w^T).
- Fusion head in single-pass float32r.
"""

import sys
import types

sys.path.insert(0, "/opt/trn_rl_repo")

import numpy as np
import ml_dtypes

import concourse.bacc as bacc
import concourse.mybir as mybir
import concourse.tile as tile
from concourse.bass_utils import run_bass_kernel_spmd
from concourse.masks import make_identity

F32 = mybir.dt.float32
F32R = mybir.dt.float32r
BF16 = mybir.dt.bfloat16
AF = mybir.ActivationFunctionType
ALU = mybir.AluOpType
AX = mybir.AxisListType

NCORES = 8
B = 16384
BC = B // NCORES  # 2048 per core
D = 1024
H = 2048
E = 8
EPS = 1e-5
P = 128
CHUNK = 1024  # batch columns processed per device-side chunk
NCHUNK = BC // CHUNK
BT = 512  # matmul moving-dim tile


def _q_f32r(x):
    """Round-half-up to float32r (12 low mantissa bits dropped) - matches HW."""
    x = np.ascontiguousarray(x, dtype=np.float32)
    xi = x.view(np.uint32)
    out = ((xi + np.uint32(1 << 11)) & (np.uint32(0xFFFFFFFF) << np.uint32(12))).view(
        np.float32
    )
    return out.reshape(x.shape).copy()


def _split_f32r(x):
    xh = _q_f32r(x)
    xl = _q_f32r(np.asarray(x, np.float32) - xh)
    return xh, xl


def _bias_tiles(b):
    """[n] -> [128, n//128]: column t = b[t*128:(t+1)*128] (per-partition bias)."""
    n = b.shape[0]
    return np.ascontiguousarray(b.reshape(n // P, P).T, dtype=np.float32)


def _build_program():
    nc = bacc.Bacc(None, target_bir_lowering=False)

    def din(name, shape, dt):
        return nc.dram_tensor(name, list(shape), dt, kind="ExternalInput")

    x1h = din("x1h", [D, BC], F32R)
    x1l = din("x1l", [D, BC], F32R)
    x2h = din("x2h", [D, BC], F32R)
    x2l = din("x2l", [D, BC], F32R)
    p1h = din("p1h", [D, D], F32R)
    p1l = din("p1l", [D, D], F32R)
    p2h = din("p2h", [D, D], F32R)
    p2l = din("p2l", [D, D], F32R)
    m1h = din("m1h", [2 * D, H], F32R)
    m1l = din("m1l", [2 * D, H], F32R)
    m2h = din("m2h", [H, H], F32R)
    m2l = din("m2l", [H, H], F32R)
    m3h = din("m3h", [H, 2 * D], F32R)
    m3l = din("m3l", [H, 2 * D], F32R)
    pb1 = din("pb1", [P, D // P], F32)
    pb2 = din("pb2", [P, D // P], F32)
    mb1 = din("mb1", [P, H // P], F32)
    mb2 = din("mb2", [P, H // P], F32)
    mb3 = din("mb3", [P, 2 * D // P], F32)
    lng = din("lng", [P, 2 * D // P], F32)
    lnb = din("lnb", [P, 2 * D // P], F32)
    gwt = din("gwt", [D, E], F32)
    gbb = din("gbb", [P, E], F32)
    exw = din("exw", [E, D, D], BF16)
    exb = din("exb", [E, D], BF16)
    fwt = din("fwt", [2 * D, D], BF16)
    fb = din("fb", [P, D // P], F32)
    rwt = din("rwt", [D, 1], F32)
    rb = din("rb", [1, 1], F32)

    out = nc.dram_tensor("out", [1, BC], F32, kind="ExternalOutput")

    from contextlib import ExitStack

    with tile.TileContext(nc) as tc, ExitStack() as ctx:
        const = ctx.enter_context(tc.tile_pool(name="const", bufs=1))
        act = ctx.enter_context(tc.tile_pool(name="act", bufs=1))
        tp = ctx.enter_context(tc.tile_pool(name="tp", bufs=2))
        tpk = ctx.enter_context(tc.tile_pool(name="tpk", bufs=2))
        wp = ctx.enter_context(tc.tile_pool(name="wp", bufs=2))
        bc8 = ctx.enter_context(tc.tile_pool(name="bc8", bufs=4))
        sp = ctx.enter_context(tc.tile_pool(name="sp", bufs=4))
        rows = ctx.enter_context(tc.tile_pool(name="rows", bufs=4))
        ps = ctx.enter_context(tc.tile_pool(name="ps", bufs=4, space="PSUM"))
        dram = ctx.enter_context(tc.tile_pool(name="dram", bufs=4, space="DRAM"))

        # constants
        ones_col = const.tile([P, 1], F32)
        nc.vector.memset(ones_col, 1.0)
        ones_row = const.tile([1, P], F32)
        nc.vector.memset(ones_row, 1.0)
        eps_row = const.tile([1, 1], F32)
        nc.vector.memset(eps_row, EPS)
        ident_bf = const.tile([P, P], BF16)
        make_identity(nc, ident_bf)
        gwt_sb = const.tile([P, D // P, E], F32)
        nc.sync.dma_start(gwt_sb, gwt.rearrange("(kt p) e -> p kt e", p=P))
        gbb_sb = const.tile([P, E], F32)
        nc.sync.dma_start(gbb_sb, gbb[:, :])
        lng_sb = const.tile([P, 2 * D // P], F32)
        nc.sync.dma_start(lng_sb, lng[:, :])
        lnb_sb = const.tile([P, 2 * D // P], F32)
        nc.sync.dma_start(lnb_sb, lnb[:, :])
        exb_sb = const.tile([E, D], BF16)
        nc.sync.dma_start(exb_sb, exb[:, :])
        rwt_sb = const.tile([P, D // P, 1], F32)
        nc.sync.dma_start(rwt_sb, rwt.rearrange("(kt p) o -> p kt o", p=P))
        rb_sb = const.tile([1, 1], F32)
        nc.sync.dma_start(rb_sb, rb[:, :])
        bias_sbs = {}
        for name, t, nt in [
            ("pb1", pb1, D // P),
            ("pb2", pb2, D // P),
            ("mb1", mb1, H // P),
            ("mb2", mb2, H // P),
            ("mb3", mb3, 2 * D // P),
            ("fb", fb, D // P),
        ]:
            s = const.tile([P, nt], F32, name=f"bias_{name}", tag=f"bias_{name}")
            nc.sync.dma_start(s, t[:, :])
            bias_sbs[name] = s

        wt_dram = [
            dram.tile([E, BC], BF16, name=f"wt_dram{i}", tag=f"wt_dram{i}")
            for i in range(2)
        ]

        def split_layer(
            in_provider,
            w_hi,
            w_lo,
            kt,
            ot,
            bias_sb,
            relu,
            out_tile,
            out_ft_off,
            ccols,
        ):
            """One split-3 f32r layer: out[:, ot tiles, ccols] over CHUNK cols.

            in_provider(k, g) -> (kh, kl) SBUF tiles [P, CHUNK] f32r.
            w_hi/w_lo: DRAM [kt*P, ot*P]. psum: groups of 4 dout tiles,
            each [P, CHUNK] (2 banks).
            """
            ngroups = ot // 4
            for g in range(ngroups):
                psums = [
                    ps.tile([P, CHUNK], F32, name=f"mmps{i}", tag="ps")
                    for i in range(4)
                ]
                for k in range(kt):
                    kh, kl = in_provider(k, g)
                    wg_h = wp.tile([P, 4 * P], F32R, tag="wg_h")
                    nc.sync.dma_start(
                        wg_h, w_hi[k * P : (k + 1) * P, g * 4 * P : (g + 1) * 4 * P]
                    )
                    wg_l = wp.tile([P, 4 * P], F32R, tag="wg_l")
                    nc.sync.dma_start(
                        wg_l, w_lo[k * P : (k + 1) * P, g * 4 * P : (g + 1) * 4 * P]
                    )
                    for dg in range(4):
                        wh_t = wg_h[:, dg * P : (dg + 1) * P]
                        wl_t = wg_l[:, dg * P : (dg + 1) * P]
                        for bt in range(CHUNK // BT):
                            sl = slice(bt * BT, (bt + 1) * BT)
                            first = k == 0
                            last = k == kt - 1
                            nc.tensor.matmul(
                                psums[dg][:, sl], wh_t, kh[:, sl],
                                start=first, stop=False,
                            )
                            nc.tensor.matmul(
                                psums[dg][:, sl], wh_t, kl[:, sl],
                                start=False, stop=False,
                            )
                            nc.tensor.matmul(
                                psums[dg][:, sl], wl_t, kh[:, sl],
                                start=False, stop=last,
                            )
                for dg in range(4):
                    dout = g * 4 + dg
                    for bt in range(CHUNK // BT):
                        sl = slice(bt * BT, (bt + 1) * BT)
                        nc.scalar.activation(
                            out_tile[:, out_ft_off + dout, sl],
                            psums[dg][:, sl],
                            AF.Relu if relu else AF.Identity,
                            bias=bias_sb[:, dout : dout + 1],
                            scale=1.0,
                        )

        def sbuf_split_provider(src_tile):
            """Split f32 activation tiles [P, ft, CHUNK] on the fly."""

            def provider(k, g):
                kh = tpk.tile([P, CHUNK], F32R, tag="kh")
                nc.gpsimd.dma_start(kh, src_tile[:, k, :])
                kl = tpk.tile([P, CHUNK], F32R, tag="kl")
                nc.vector.tensor_tensor(
                    kl, src_tile[:, k, :], kh.bitcast(F32), ALU.subtract
                )
                return kh, kl

            return provider

        def dram_pair_provider(xh_d, xl_d, ccols):
            def provider(k, g):
                kh = tpk.tile([P, CHUNK], F32R, tag="kh")
                nc.sync.dma_start(kh, xh_d[k * P : (k + 1) * P, ccols])
                kl = tpk.tile([P, CHUNK], F32R, tag="kl")
                nc.sync.dma_start(kl, xl_d[k * P : (k + 1) * P, ccols])
                return kh, kl

            return provider

        for c in range(NCHUNK):
            ccols = slice(c * CHUNK, (c + 1) * CHUNK)

            # ---- projections -> h [2048 feats, CHUNK] f32
            h = act.tile([P, 16, CHUNK], F32, tag="h")
            split_layer(
                dram_pair_provider(x1h, x1l, ccols), p1h, p1l, D // P, D // P,
                bias_sbs["pb1"], False, h, 0, ccols,
            )
            split_layer(
                dram_pair_provider(x2h, x2l, ccols), p2h, p2l, D // P, D // P,
                bias_sbs["pb2"], False, h, 8, ccols,
            )

            # ---- m1, m2 (relu), m3 (no relu)
            h1 = act.tile([P, 16, CHUNK], F32, tag="h1")
            split_layer(
                sbuf_split_provider(h), m1h, m1l, 16, 16,
                bias_sbs["mb1"], True, h1, 0, ccols,
            )
            h2 = act.tile([P, 16, CHUNK], F32, tag="h")
            split_layer(
                sbuf_split_provider(h1), m2h, m2l, 16, 16,
                bias_sbs["mb2"], True, h2, 0, ccols,
            )
            a = act.tile([P, 16, CHUNK], F32, tag="h1")
            split_layer(
                sbuf_split_provider(h2), m3h, m3l, 16, 16,
                bias_sbs["mb3"], False, a, 0, ccols,
            )

            # ---- LayerNorm over the 2048 features (partition axis, via matmul)
            xm = act.tile([P, 16, CHUNK], F32, tag="h")
            for bt in range(CHUNK // BT):
                sl = slice(bt * BT, (bt + 1) * BT)
                ps_ln = ps.tile([P, CHUNK], F32, name="ps_ln", tag="ps")
                ps_sum = ps_ln[0:1, 0:BT]
                ps_sq = ps_ln[0:1, BT : 2 * BT]
                for ft in range(16):
                    at = a[:, ft, sl]
                    sq = tp.tile([P, BT], F32, tag="lnt")
                    nc.vector.tensor_tensor(sq, at, at, ALU.mult)
                    nc.tensor.matmul(
                        ps_sum, ones_col, at, start=(ft == 0), stop=(ft == 15)
                    )
                    nc.tensor.matmul(
                        ps_sq, ones_col, sq, start=(ft == 0), stop=(ft == 15)
                    )
                mean = rows.tile([1, BT], F32, tag="lnrow")
                nc.vector.tensor_scalar(mean, ps_sum, 1.0 / (2 * D), None, ALU.mult)
                e2 = rows.tile([1, BT], F32, tag="lnrow")
                nc.vector.tensor_scalar(e2, ps_sq, 1.0 / (2 * D), None, ALU.mult)
                nm2 = rows.tile([1, BT], F32, tag="lnrow")
                nc.vector.scalar_tensor_tensor(
                    nm2, mean, -1.0, mean, ALU.mult, ALU.mult
                )  # -mean^2
                var = rows.tile([1, BT], F32, tag="lnrow")
                nc.vector.tensor_tensor(var, e2, nm2, ALU.add)
                sd = rows.tile([1, BT], F32, tag="lnrow")
                nc.scalar.activation(sd, var, AF.Sqrt, bias=eps_row, scale=1.0)
                rstd = rows.tile([1, BT], F32, tag="lnrow")
                nc.vector.reciprocal(rstd, sd)
                trow = rows.tile([1, BT], F32, tag="lnrow")
                nc.vector.scalar_tensor_tensor(
                    trow, mean, -1.0, rstd, ALU.mult, ALU.mult
                )
                # broadcast via DRAM round-trip
                s_d = dram.tile([1, BT], F32, name="s_d", tag="s_d")
                t_d = dram.tile([1, BT], F32, name="t_d", tag="t_d")
                nc.sync.dma_start(s_d[:], rstd)
                nc.sync.dma_start(t_d[:], trow)
                sbc = tp.tile([P, BT], F32, tag="sbc")
                nc.gpsimd.dma_start(sbc, s_d[0:1, :].to_broadcast([P, BT]))
                tbc = tp.tile([P, BT], F32, tag="tbc")
                nc.gpsimd.dma_start(tbc, t_d[0:1, :].to_broadcast([P, BT]))
                for ft in range(16):
                    t1 = tp.tile([P, BT], F32, tag="lnt")
                    nc.vector.tensor_tensor(t1, a[:, ft, sl], sbc, ALU.mult)
                    nc.vector.tensor_tensor(t1, t1, tbc, ALU.add)
                    nc.vector.tensor_scalar(
                        xm[:, ft, sl], t1,
                        lng_sb[:, ft : ft + 1], lnb_sb[:, ft : ft + 1],
                        ALU.mult, ALU.add,
                    )

            # ---- gate + top2 softmax weights (fp32, exact routing)
            wts_sb = [None, None]
            for br in range(2):
                wts = act.tile([E, CHUNK], BF16, tag=f"wts{br}")
                wts_sb[br] = wts
                for bs in range(CHUNK // P):
                    bsl = slice(bs * P, (bs + 1) * P)
                    ps_g = ps.tile([P, CHUNK], F32, name="ps_g", tag="ps")
                    psg = ps_g[:, 0:E]
                    for k in range(8):
                        nc.tensor.matmul(
                            psg, xm[:, br * 8 + k, bsl], gwt_sb[:, k, :],
                            start=(k == 0), stop=(k == 7),
                        )
                    lg = sp.tile([P, E], F32, tag="lg")
                    nc.vector.scalar_tensor_tensor(
                        lg, psg, 1.0, gbb_sb, ALU.mult, ALU.add
                    )
                    mx1 = sp.tile([P, 1], F32, tag="mx1")
                    nc.vector.reduce_max(mx1, lg, axis=AX.X)
                    cmp = sp.tile([P, E], F32, tag="cmp")
                    nc.vector.tensor_scalar(cmp, lg, mx1, -1e30, ALU.is_ge, ALU.mult)
                    masked = sp.tile([P, E], F32, tag="masked")
                    nc.vector.tensor_tensor(masked, lg, cmp, ALU.add)
                    mx2 = sp.tile([P, 1], F32, tag="mx2")
                    nc.vector.reduce_max(mx2, masked, axis=AX.X)
                    negm1 = sp.tile([P, 1], F32, tag="negm1")
                    nc.vector.tensor_scalar(negm1, mx1, -1.0, None, ALU.mult)
                    ex = sp.tile([P, E], F32, tag="ex")
                    nc.scalar.activation(ex, lg, AF.Exp, bias=negm1, scale=1.0)
                    sel = sp.tile([P, E], F32, tag="sel")
                    nc.vector.tensor_scalar(sel, lg, mx2, None, ALU.is_ge)
                    wu = sp.tile([P, E], F32, tag="wu")
                    nc.vector.tensor_tensor(wu, ex, sel, ALU.mult)
                    den = sp.tile([P, 1], F32, tag="den")
                    nc.vector.reduce_sum(den, wu, axis=AX.X)
                    rec = sp.tile([P, 1], F32, tag="rec")
                    nc.vector.reciprocal(rec, den)
                    wbf = sp.tile([P, E], BF16, tag="wbf")
                    nc.vector.tensor_scalar(wbf, wu, rec, None, ALU.mult)
                    ps_t = ps.tile([P, 2 * CHUNK], BF16, name="ps_t", tag="ps")
                    pst = ps_t[0:E, 0:P]
                    nc.tensor.transpose(pst, wbf, ident_bf)
                    nc.vector.tensor_copy(wts[:, bsl], pst)
                nc.sync.dma_start(wt_dram[br][:, ccols], wts)

            # ---- dense MoE (bf16) fused with the fusion head, per bt stripe
            fust = act.tile([P, 8, CHUNK], F32, tag="h1")
            for bt in range(CHUNK // BT):
                sl = slice(bt * BT, (bt + 1) * BT)
                moebt = act.tile([P, 16, BT], BF16, tag="moebt")
                for br in range(2):
                    mt = [
                        ps.tile([P, CHUNK], F32, name=f"mops{i}", tag="ps")
                        for i in range(4)
                    ]
                    psums = [
                        mt[i // 2][:, (i % 2) * BT : (i % 2 + 1) * BT]
                        for i in range(8)
                    ]
                    wbcs = []
                    for e in range(E):
                        wbc = bc8.tile([P, BT], BF16, tag="wbc")
                        nc.gpsimd.dma_start(
                            wbc,
                            wt_dram[br][e : e + 1, c * CHUNK + bt * BT :
                                        c * CHUNK + (bt + 1) * BT].to_broadcast(
                                [P, BT]
                            ),
                        )
                        wbcs.append(wbc)
                    for e in range(E):
                        wbc = wbcs[e]
                        for k in range(8):
                            xw = tp.tile([P, BT], BF16, tag="xw")
                            nc.vector.tensor_tensor(
                                xw, xm[:, br * 8 + k, sl], wbc, ALU.mult
                            )
                            wexp = wp.tile([P, D], BF16, tag="wexp")
                            nc.sync.dma_start(
                                wexp, exw[e, k * P : (k + 1) * P, :]
                            )
                            for dout in range(8):
                                nc.tensor.matmul(
                                    psums[dout],
                                    wexp[:, dout * P : (dout + 1) * P],
                                    xw,
                                    start=(e == 0 and k == 0),
                                    stop=False,
                                )
                    for dout in range(8):
                        nc.tensor.matmul(
                            psums[dout],
                            exb_sb[:, dout * P : (dout + 1) * P],
                            wts_sb[br][:, sl],
                            start=False,
                            stop=True,
                        )
                        nc.vector.tensor_copy(
                            moebt[:, br * 8 + dout, :], psums[dout]
                        )
                # fusion head for this bt stripe (bf16)
                ft_t = [
                    ps.tile([P, CHUNK], F32, name=f"fps{i}", tag="ps")
                    for i in range(4)
                ]
                psf = [
                    ft_t[i // 2][:, (i % 2) * BT : (i % 2 + 1) * BT]
                    for i in range(8)
                ]
                for k in range(16):
                    wgf = wp.tile([P, D], BF16, tag="wexp")
                    nc.sync.dma_start(wgf, fwt[k * P : (k + 1) * P, :])
                    for dout in range(8):
                        nc.tensor.matmul(
                            psf[dout],
                            wgf[:, dout * P : (dout + 1) * P],
                            moebt[:, k, :],
                            start=(k == 0),
                            stop=(k == 15),
                        )
                for dout in range(8):
                    nc.scalar.activation(
                        fust[:, dout, sl], psf[dout], AF.Identity,
                        bias=bias_sbs["fb"][:, dout : dout + 1], scale=1.0,
                    )

            for bt in range(CHUNK // BT):
                sl = slice(bt * BT, (bt + 1) * BT)
                ps_r = ps.tile([P, CHUNK], F32, name="ps_r", tag="ps")
                psr = ps_r[0:1, 0:BT]
                for k in range(8):
                    nc.tensor.matmul(
                        psr, rwt_sb[:, k, :], fust[:, k, sl],
                        start=(k == 0), stop=(k == 7),
                    )
                orow = rows.tile([1, BT], F32, tag="lnrow")
                nc.vector.tensor_scalar(orow, psr, rb_sb[0:1, 0:1], None, ALU.add)
                nc.sync.dma_start(out[0:1, c * CHUNK + bt * BT :
                                      c * CHUNK + (bt + 1) * BT], orow)

    nc.compile()
    return nc


_NC_CACHE = None


def _get_program():
    global _NC_CACHE
    if _NC_CACHE is None:
        _NC_CACHE = _build_program()
    return _NC_CACHE


def _host_prep_shared(inp):
    """Weight folding/transposition/splitting shared across all cores."""
    f = np.float32
    g1 = np.asarray(inp["bn1_g"], f) / np.sqrt(f(1.0) + f(EPS))
    g2 = np.asarray(inp["bn2_g"], f) / np.sqrt(f(1.0) + f(EPS))
    gf = np.asarray(inp["bnf_g"], f) / np.sqrt(f(1.0) + f(EPS))

    m1w = (np.asarray(inp["m1_w"], f) * g1[:, None]).T  # [2D, H]
    m1b = np.asarray(inp["m1_b"], f) * g1 + np.asarray(inp["bn1_b"], f)
    m2w = (np.asarray(inp["m2_w"], f) * g2[:, None]).T  # [H, H]
    m2b = np.asarray(inp["m2_b"], f) * g2 + np.asarray(inp["bn2_b"], f)
    fww = (np.asarray(inp["fus_w"], f) * gf[:, None]).T  # [2D, D]
    fbb = np.asarray(inp["fus_b"], f) * gf + np.asarray(inp["bnf_b"], f)

    shared = {}
    for nm, w in [
        ("p1", np.asarray(inp["p1_w"], f).T),
        ("p2", np.asarray(inp["p2_w"], f).T),
        ("m1", m1w),
        ("m2", m2w),
        ("m3", np.asarray(inp["m3_w"], f).T),
    ]:
        hi, lo = _split_f32r(np.ascontiguousarray(w))
        shared[nm + "h"] = hi
        shared[nm + "l"] = lo
    shared["pb1"] = _bias_tiles(np.asarray(inp["p1_b"], f))
    shared["pb2"] = _bias_tiles(np.asarray(inp["p2_b"], f))
    shared["mb1"] = _bias_tiles(m1b)
    shared["mb2"] = _bias_tiles(m2b)
    shared["mb3"] = _bias_tiles(np.asarray(inp["m3_b"], f))
    shared["lng"] = _bias_tiles(np.asarray(inp["ln_g"], f))
    shared["lnb"] = _bias_tiles(np.asarray(inp["ln_b"], f))
    shared["gwt"] = np.ascontiguousarray(np.asarray(inp["gate_w"], f).T)
    shared["gbb"] = np.ascontiguousarray(
        np.broadcast_to(np.asarray(inp["gate_b"], f)[None, :], (P, E))
    )
    exw = np.asarray(inp["ex_w"], f)  # [E, out, in]
    shared["exw"] = np.ascontiguousarray(
        np.transpose(exw, (0, 2, 1))
    ).astype(ml_dtypes.bfloat16)
    shared["exb"] = np.asarray(inp["ex_b"], f).astype(ml_dtypes.bfloat16)
    shared["fwt"] = np.ascontiguousarray(fww).astype(ml_dtypes.bfloat16)
    shared["fb"] = _bias_tiles(fbb)
    shared["rwt"] = np.ascontiguousarray(np.asarray(inp["reg_w"], f).T)
    shared["rb"] = np.asarray(inp["reg_b"], f).reshape(1, 1)
    return shared


def kernel(**inputs):
    nc = _get_program()
    shared = _host_prep_shared(inputs)

    x1 = np.asarray(inputs["x1"], np.float32)
    x2 = np.asarray(inputs["x2"], np.float32)

    in_maps = []
    for core in range(NCORES):
        rows_sl = slice(core * BC, (core + 1) * BC)
        x1t = np.ascontiguousarray(x1[rows_sl].T)  # [D, BC]
        x2t = np.ascontiguousarray(x2[rows_sl].T)
        x1th, x1tl = _split_f32r(x1t)
        x2th, x2tl = _split_f32r(x2t)
        m = dict(shared)
        m["x1h"], m["x1l"] = x1th, x1tl
        m["x2h"], m["x2l"] = x2th, x2tl
        in_maps.append(m)

    kwargs = {}
    if globals().get("TRACE"):
        kwargs["trace"] = True
    res = run_bass_kernel_spmd(nc, in_maps, core_ids=list(range(NCORES)), **kwargs)
    globals()["_LAST_RESULT"] = res
    out = np.concatenate([r["out"][0] for r in res.results])
    return out.reshape(B, 1).astype(np.float32)


if __name__ == "__main__":
    rng = np.random.default_rng(0)
    fake = {"x1": rng.standard_normal((B, D), dtype=np.float32)}
    print("kernel module loaded; use test.py to validate")

